# revision 1
# baseline (speedup 1.0000x reference)
"""Trainium2 Bass kernel: Bahdanau-style additive attention (nn_Attention).

Reference computation (per batch b):
    q = queries[:, b, :] @ Wq.T          # (Tq, C)   C = N_ATT = 128
    k = keys[:, b, :] @ Wk.T             # (Tk, C)
    v = values[:, b, :] @ Wv.T           # (Tk, C)
    logits[q, k] = sum_c Wvec[c] * tanh(q[q, c] + k[k, c])
    attn = softmax_k(logits)             # (Tq, Tk)
    out  = (attn @ v).T                  # (C, Tq)
returns (out (B, C, Tq), attn (B, Tq, Tk))

Sharding: data-parallel over batch B=8, one batch per NeuronCore (8 cores).

Per-core dataflow (all layouts chosen so C=128 sits on partitions):
  - PE transposes inputs, computes qproj (C, Tq), kproj (C, Tk), vprojT (Tk, C)
  - VectorE: S[c, (k, q)] = qproj[c, q] + kproj[c, k]   (per-partition scalar add)
  - ScalarE: E = tanh(S) in one giant activation per k-group (output fp16)
  - PE: logits built directly in (q, k) layout: for each (k, q-chunk) one
    matmul with lhsT = E-slice (c, 128q) and rhs = Wvec (c, 1) -> PSUM column k
  - softmax over free dim k, DMA attn out, PE-transpose attn, final matmul
    out = vprojT.T @ attnT
"""

import numpy as np

import concourse.bacc as bacc
import concourse.bass as bass
import concourse.mybir as mybir
from concourse import masks, tile

F32 = mybir.dt.float32
F16 = mybir.dt.float16
AF = mybir.ActivationFunctionType
AX = mybir.AxisListType

# Problem shapes (hardcoded per the harness contract).
TQ, B, TK = 512, 8, 256
NMEL, NCTX, C = 80, 640, 128
P = 128
IC = NCTX // P  # 5 contraction chunks for the 640-dim projections
QN = TQ // P    # 4 query chunks
KN = TK // P    # 2 key chunks
G = 8           # k-group size per tanh mega-instruction
NG = TK // G
N_CORES = 8


def _emit(tc, nc, q_d, k_d, v_d, wq_d, wk_d, wv_d, wvec_d, out_d, attn_d):
    with (
        tc.tile_pool(name="const", bufs=1) as cpool,
        tc.tile_pool(name="spool", bufs=2) as spool,
        tc.tile_pool(name="epool", bufs=2) as epool,
        tc.tile_pool(name="smx", bufs=2) as smx,
        tc.tile_pool(name="psT", bufs=2, space=bass.MemorySpace.PSUM) as psT,
        tc.tile_pool(name="psP", bufs=2, space=bass.MemorySpace.PSUM) as psP,
        tc.tile_pool(name="psA", bufs=1, space=bass.MemorySpace.PSUM) as psA,
    ):
        ident = cpool.tile([P, P], F32)
        masks.make_identity(nc, ident[:])

        wvec = cpool.tile([P, 1], F16)
        nc.sync.dma_start(wvec[:], wvec_d[:])

        # ---------------- queries -> qproj (C, TQ) ----------------
        q_in = cpool.tile([P, QN, NMEL], F32)
        nc.sync.dma_start(q_in[:], q_d.ap().rearrange("(n p) i -> p n i", p=P))
        wq_in = cpool.tile([P, NMEL], F32)
        nc.sync.dma_start(wq_in[:], wq_d[:])

        qT = cpool.tile([NMEL, TQ], F32)
        for n in range(QN):
            tp_q = psT.tile([NMEL, P], F32, tag="tp", name=f"tp_q{n}")
            nc.tensor.transpose(tp_q[:], q_in[:, n, :], ident[:])
            nc.vector.tensor_copy(qT[:, n * P:(n + 1) * P], tp_q[:])
        wqT = cpool.tile([NMEL, C], F32)
        tp_wq = psT.tile([NMEL, P], F32, tag="tp")
        nc.tensor.transpose(tp_wq[:], wq_in[:], ident[:])
        nc.vector.tensor_copy(wqT[:], tp_wq[:])

        qproj_ps = psP.tile([C, TQ], F32, tag="pp", name="qproj_ps")
        nc.tensor.matmul(qproj_ps[:], wqT[:], qT[:], start=True, stop=True)
        qproj = cpool.tile([C, TQ], F32)
        nc.vector.tensor_copy(qproj[:], qproj_ps[:])

        # ---------------- keys -> kproj (C, TK) ----------------
        k_in = cpool.tile([P, KN, NCTX], F32)
        nc.sync.dma_start(k_in[:], k_d.ap().rearrange("(n p) i -> p n i", p=P))
        wk_in = cpool.tile([P, NCTX], F32)
        nc.sync.dma_start(wk_in[:], wk_d[:])

        kT = cpool.tile([P, IC, TK], F32)
        for ic in range(IC):
            for n in range(KN):
                tp_k = psT.tile([P, P], F32, tag="tp", name=f"tp_k{ic}_{n}")
                nc.tensor.transpose(tp_k[:], k_in[:, n, ic * P:(ic + 1) * P], ident[:])
                nc.vector.tensor_copy(kT[:, ic, n * P:(n + 1) * P], tp_k[:])
        wkT = cpool.tile([P, IC, C], F32)
        for ic in range(IC):
            tp_wk = psT.tile([P, P], F32, tag="tp", name=f"tp_wk{ic}")
            nc.tensor.transpose(tp_wk[:], wk_in[:, ic * P:(ic + 1) * P], ident[:])
            nc.vector.tensor_copy(wkT[:, ic, :], tp_wk[:])

        kproj_ps = psP.tile([C, TK], F32, tag="pp", name="kproj_ps")
        for ic in range(IC):
            nc.tensor.matmul(kproj_ps[:], wkT[:, ic, :], kT[:, ic, :],
                             start=(ic == 0), stop=(ic == IC - 1))
        kproj = cpool.tile([C, TK], F32)
        nc.vector.tensor_copy(kproj[:], kproj_ps[:])

        # ---------------- values -> vprojT (TK, C) ----------------
        v_in = cpool.tile([P, KN, NCTX], F32)
        nc.sync.dma_start(v_in[:], v_d.ap().rearrange("(n p) i -> p n i", p=P))
        wv_in = cpool.tile([P, NCTX], F32)
        nc.sync.dma_start(wv_in[:], wv_d[:])

        vT = cpool.tile([P, IC, TK], F32)
        for ic in range(IC):
            for n in range(KN):
                tp_v = psT.tile([P, P], F32, tag="tp", name=f"tp_v{ic}_{n}")
                nc.tensor.transpose(tp_v[:], v_in[:, n, ic * P:(ic + 1) * P], ident[:])
                nc.vector.tensor_copy(vT[:, ic, n * P:(n + 1) * P], tp_v[:])
        wvT = cpool.tile([P, IC, C], F32)
        for ic in range(IC):
            tp_wv = psT.tile([P, P], F32, tag="tp", name=f"tp_wv{ic}")
            nc.tensor.transpose(tp_wv[:], wv_in[:, ic * P:(ic + 1) * P], ident[:])
            nc.vector.tensor_copy(wvT[:, ic, :], tp_wv[:])

        vprojT = cpool.tile([P, KN, C], F32)
        for kc in range(KN):
            vp_ps = psP.tile([P, C], F32, tag="pp", name=f"vp_ps{kc}")
            for ic in range(IC):
                nc.tensor.matmul(vp_ps[:], vT[:, ic, kc * P:(kc + 1) * P],
                                 wvT[:, ic, :], start=(ic == 0), stop=(ic == IC - 1))
            nc.vector.tensor_copy(vprojT[:, kc, :], vp_ps[:])

        # ---------------- energy + logits ----------------
        # attn_ps0 holds q-chunks 0,1 ; attn_ps1 holds q-chunks 2,3
        attn_ps0 = psA.tile([P, 2 * TK], F32)
        attn_ps1 = psA.tile([P, 2 * TK], F32)

        for g in range(NG):
            S = spool.tile([P, G * TQ], F32, tag="S", name=f"S{g}")
            for j in range(G):
                k = g * G + j
                nc.vector.tensor_scalar_add(
                    S[:, j * TQ:(j + 1) * TQ], qproj[:], kproj[:, k:k + 1])
            E = epool.tile([P, G * TQ], F16, tag="E", name=f"E{g}")
            nc.scalar.activation(E[:], S[:], AF.Tanh)
            for j in range(G):
                k = g * G + j
                for qc in range(4):
                    ps = attn_ps0 if qc < 2 else attn_ps1
                    col = (qc % 2) * TK + k
                    nc.tensor.matmul(
                        ps[:, col:col + 1],
                        E[:, j * TQ + qc * P: j * TQ + (qc + 1) * P],
                        wvec[:], start=True, stop=True)

        # ---------------- softmax (over k) + attn out + transpose ----------------
        attnT = cpool.tile([P, KN, TQ], F32)
        for qc in range(4):
            ps = attn_ps0 if qc < 2 else attn_ps1
            asl = ps[:, (qc % 2) * TK:(qc % 2 + 1) * TK]
            nmx = smx.tile([P, 1], F32, tag="nmx", name=f"nmx{qc}")
            nc.vector.reduce_max(nmx[:], asl, axis=AX.X, negate=True)
            ae = smx.tile([P, TK], F32, tag="ae", name=f"ae{qc}")
            nc.scalar.activation(ae[:], asl, AF.Exp, bias=nmx[:])
            sm = smx.tile([P, 1], F32, tag="sm", name=f"sm{qc}")
            nc.vector.reduce_sum(sm[:], ae[:], axis=AX.X)
            rc = smx.tile([P, 1], F32, tag="rc", name=f"rc{qc}")
            nc.vector.reciprocal(rc[:], sm[:])
            an = smx.tile([P, TK], F32, tag="an", name=f"an{qc}")
            nc.vector.tensor_scalar_mul(an[:], ae[:], rc[:])
            nc.sync.dma_start(attn_d[qc * P:(qc + 1) * P, :], an[:])
            for kc in range(KN):
                tp_a = psT.tile([P, P], F32, tag="tp", name=f"tp_a{qc}_{kc}")
                nc.tensor.transpose(tp_a[:], an[:, kc * P:(kc + 1) * P], ident[:])
                nc.vector.tensor_copy(attnT[:, kc, qc * P:(qc + 1) * P], tp_a[:])

        # ---------------- out = vprojT.T @ attnT  (C, TQ) ----------------
        out_ps = psP.tile([C, TQ], F32, tag="pp", name="out_ps")
        for kc in range(KN):
            nc.tensor.matmul(out_ps[:], vprojT[:, kc, :], attnT[:, kc, :],
                             start=(kc == 0), stop=(kc == KN - 1))
        out_sb = cpool.tile([C, TQ], F32)
        nc.vector.tensor_copy(out_sb[:], out_ps[:])
        nc.sync.dma_start(out_d[:], out_sb[:])


def build_nc():
    nc = bacc.Bacc("TRN2", target_bir_lowering=False, debug=False,
                   enable_asserts=False, num_devices=N_CORES)
    q_d = nc.dram_tensor("queries", [TQ, NMEL], F32, kind="ExternalInput")
    k_d = nc.dram_tensor("keys", [TK, NCTX], F32, kind="ExternalInput")
    v_d = nc.dram_tensor("values", [TK, NCTX], F32, kind="ExternalInput")
    wq_d = nc.dram_tensor("Wq", [C, NMEL], F32, kind="ExternalInput")
    wk_d = nc.dram_tensor("Wk", [C, NCTX], F32, kind="ExternalInput")
    wv_d = nc.dram_tensor("Wv", [C, NCTX], F32, kind="ExternalInput")
    wvec_d = nc.dram_tensor("wvec16", [C, 1], F16, kind="ExternalInput")
    out_d = nc.dram_tensor("out", [C, TQ], F32, kind="ExternalOutput")
    attn_d = nc.dram_tensor("attn", [TQ, TK], F32, kind="ExternalOutput")

    with tile.TileContext(nc) as tc:
        _emit(tc, nc, q_d, k_d, v_d, wq_d, wk_d, wv_d, wvec_d, out_d, attn_d)
    nc.compile()
    return nc


_NC = None


def _get_nc():
    global _NC
    if _NC is None:
        _NC = build_nc()
    return _NC


def make_in_maps(queries, keys, values, Wq, Wk, Wv, Wvec):
    wvec16 = np.ascontiguousarray(np.asarray(Wvec).astype(np.float16).reshape(C, 1))
    wq = np.ascontiguousarray(np.asarray(Wq, dtype=np.float32))
    wk = np.ascontiguousarray(np.asarray(Wk, dtype=np.float32))
    wv = np.ascontiguousarray(np.asarray(Wv, dtype=np.float32))
    in_maps = []
    for b in range(B):
        in_maps.append({
            "queries": np.ascontiguousarray(np.asarray(queries)[:, b, :], dtype=np.float32),
            "keys": np.ascontiguousarray(np.asarray(keys)[:, b, :], dtype=np.float32),
            "values": np.ascontiguousarray(np.asarray(values)[:, b, :], dtype=np.float32),
            "Wq": wq, "Wk": wk, "Wv": wv, "wvec16": wvec16,
        })
    return in_maps


def kernel(queries, keys, values, Wq, Wk, Wv, Wvec):
    from concourse.bass2jax import run_bass_via_pjrt
    nc = _get_nc()
    in_maps = make_in_maps(queries, keys, values, Wq, Wk, Wv, Wvec)
    results = run_bass_via_pjrt(nc, in_maps, n_cores=N_CORES)
    out = np.stack([results[b]["out"] for b in range(B)], axis=0)
    attn = np.stack([results[b]["attn"] for b in range(B)], axis=0)
    return out, attn


# revision 3
# speedup vs baseline: 6802.6076x; 6802.6076x over previous
"""Trainium2 Bass kernel: Bahdanau-style additive attention (nn_Attention).

Reference computation (per batch b):
    q = queries[:, b, :] @ Wq.T          # (Tq, C)   C = N_ATT = 128
    k = keys[:, b, :] @ Wk.T             # (Tk, C)
    v = values[:, b, :] @ Wv.T           # (Tk, C)
    logits[q, k] = sum_c Wvec[c] * tanh(q[q, c] + k[k, c])
    attn = softmax_k(logits)             # (Tq, Tk)
    out  = (attn @ v).T                  # (C, Tq)
returns (out (B, C, Tq), attn (B, Tq, Tk))

Sharding: data-parallel over batch B=8, one batch per NeuronCore (8 cores).

Per-core dataflow (all layouts chosen so C=128 sits on partitions):
  - PE transposes inputs, computes qproj (C, Tq), kproj (C, Tk), vprojT (Tk, C)
  - VectorE: S[c, (k, q)] = qproj[c, q] + kproj[c, k]   (per-partition scalar add)
  - ScalarE: E = tanh(S) in one giant activation per k-group (output fp16)
  - PE: logits built directly in (q, k) layout: for each (k, q-chunk) one
    matmul with lhsT = E-slice (c, 128q) and rhs = Wvec (c, 1) -> PSUM column k
  - softmax over free dim k, DMA attn out, PE-transpose attn, final matmul
    out = vprojT.T @ attnT
"""

import numpy as np

import concourse.bacc as bacc
import concourse.bass as bass
import concourse.mybir as mybir
from concourse import masks, tile

F32 = mybir.dt.float32
F16 = mybir.dt.float16
AF = mybir.ActivationFunctionType
AX = mybir.AxisListType

# Problem shapes (hardcoded per the harness contract).
TQ, B, TK = 512, 8, 256
NMEL, NCTX, C = 80, 640, 128
P = 128
IC = NCTX // P  # 5 contraction chunks for the 640-dim projections
QN = TQ // P    # 4 query chunks
KN = TK // P    # 2 key chunks
G = 8           # k-group size per tanh mega-instruction
NG = TK // G
N_CORES = 8


def _emit(tc, nc, q_d, k_d, v_d, wq_d, wk_d, wv_d, wvec_d, out_d, attn_d):
    with (
        tc.tile_pool(name="const", bufs=1) as cpool,
        tc.tile_pool(name="spool", bufs=2) as spool,
        tc.tile_pool(name="epool", bufs=2) as epool,
        tc.tile_pool(name="smx", bufs=2) as smx,
        tc.tile_pool(name="psT", bufs=2, space=bass.MemorySpace.PSUM) as psT,
        tc.tile_pool(name="psP", bufs=2, space=bass.MemorySpace.PSUM) as psP,
        tc.tile_pool(name="psA", bufs=1, space=bass.MemorySpace.PSUM) as psA,
    ):
        ident = cpool.tile([P, P], F32)
        masks.make_identity(nc, ident[:])

        wvec = cpool.tile([P, 1], F16)
        nc.sync.dma_start(wvec[:], wvec_d[:])

        # ---------------- queries -> qproj (C, TQ) ----------------
        q_in = cpool.tile([P, QN, NMEL], F32)
        nc.sync.dma_start(q_in[:], q_d.ap().rearrange("(n p) i -> p n i", p=P))
        wq_in = cpool.tile([P, NMEL], F32)
        nc.sync.dma_start(wq_in[:], wq_d[:])

        qT = cpool.tile([NMEL, TQ], F32)
        for n in range(QN):
            tp_q = psT.tile([NMEL, P], F32, tag="tp", name=f"tp_q{n}")
            nc.tensor.transpose(tp_q[:], q_in[:, n, :], ident[:])
            nc.vector.tensor_copy(qT[:, n * P:(n + 1) * P], tp_q[:])
        wqT = cpool.tile([NMEL, C], F32)
        tp_wq = psT.tile([NMEL, P], F32, tag="tp")
        nc.tensor.transpose(tp_wq[:], wq_in[:], ident[:])
        nc.vector.tensor_copy(wqT[:], tp_wq[:])

        qproj_ps = psP.tile([C, TQ], F32, tag="pp", name="qproj_ps")
        nc.tensor.matmul(qproj_ps[:], wqT[:], qT[:], start=True, stop=True)
        qproj = cpool.tile([C, TQ], F32)
        nc.vector.tensor_copy(qproj[:], qproj_ps[:])

        # ---------------- keys -> kproj (C, TK) ----------------
        k_in = cpool.tile([P, KN, NCTX], F32)
        nc.sync.dma_start(k_in[:], k_d.ap().rearrange("(n p) i -> p n i", p=P))
        wk_in = cpool.tile([P, NCTX], F32)
        nc.sync.dma_start(wk_in[:], wk_d[:])

        kT = cpool.tile([P, IC, TK], F32)
        for ic in range(IC):
            for n in range(KN):
                tp_k = psT.tile([P, P], F32, tag="tp", name=f"tp_k{ic}_{n}")
                nc.tensor.transpose(tp_k[:], k_in[:, n, ic * P:(ic + 1) * P], ident[:])
                nc.vector.tensor_copy(kT[:, ic, n * P:(n + 1) * P], tp_k[:])
        wkT = cpool.tile([P, IC, C], F32)
        for ic in range(IC):
            tp_wk = psT.tile([P, P], F32, tag="tp", name=f"tp_wk{ic}")
            nc.tensor.transpose(tp_wk[:], wk_in[:, ic * P:(ic + 1) * P], ident[:])
            nc.vector.tensor_copy(wkT[:, ic, :], tp_wk[:])

        kproj_ps = psP.tile([C, TK], F32, tag="pp", name="kproj_ps")
        for ic in range(IC):
            nc.tensor.matmul(kproj_ps[:], wkT[:, ic, :], kT[:, ic, :],
                             start=(ic == 0), stop=(ic == IC - 1))
        kproj = cpool.tile([C, TK], F32)
        nc.vector.tensor_copy(kproj[:], kproj_ps[:])

        # ---------------- values -> vprojT (TK, C) ----------------
        v_in = cpool.tile([P, KN, NCTX], F32)
        nc.sync.dma_start(v_in[:], v_d.ap().rearrange("(n p) i -> p n i", p=P))
        wv_in = cpool.tile([P, NCTX], F32)
        nc.sync.dma_start(wv_in[:], wv_d[:])

        vT = cpool.tile([P, IC, TK], F32)
        for ic in range(IC):
            for n in range(KN):
                tp_v = psT.tile([P, P], F32, tag="tp", name=f"tp_v{ic}_{n}")
                nc.tensor.transpose(tp_v[:], v_in[:, n, ic * P:(ic + 1) * P], ident[:])
                nc.vector.tensor_copy(vT[:, ic, n * P:(n + 1) * P], tp_v[:])
        wvT = cpool.tile([P, IC, C], F32)
        for ic in range(IC):
            tp_wv = psT.tile([P, P], F32, tag="tp", name=f"tp_wv{ic}")
            nc.tensor.transpose(tp_wv[:], wv_in[:, ic * P:(ic + 1) * P], ident[:])
            nc.vector.tensor_copy(wvT[:, ic, :], tp_wv[:])

        vprojT = cpool.tile([P, KN, C], F32)
        for kc in range(KN):
            vp_ps = psP.tile([P, C], F32, tag="pp", name=f"vp_ps{kc}")
            for ic in range(IC):
                nc.tensor.matmul(vp_ps[:], vT[:, ic, kc * P:(kc + 1) * P],
                                 wvT[:, ic, :], start=(ic == 0), stop=(ic == IC - 1))
            nc.vector.tensor_copy(vprojT[:, kc, :], vp_ps[:])

        # ---------------- energy + logits ----------------
        # attn_ps0 holds q-chunks 0,1 ; attn_ps1 holds q-chunks 2,3
        attn_ps0 = psA.tile([P, 2 * TK], F32)
        attn_ps1 = psA.tile([P, 2 * TK], F32)

        for g in range(NG):
            S = spool.tile([P, G * TQ], F32, tag="S", name=f"S{g}")
            for j in range(G):
                k = g * G + j
                nc.vector.tensor_scalar_add(
                    S[:, j * TQ:(j + 1) * TQ], qproj[:], kproj[:, k:k + 1])
            E = epool.tile([P, G * TQ], F16, tag="E", name=f"E{g}")
            nc.scalar.activation(E[:], S[:], AF.Tanh)
            for j in range(G):
                k = g * G + j
                for qc in range(4):
                    ps = attn_ps0 if qc < 2 else attn_ps1
                    col = (qc % 2) * TK + k
                    nc.tensor.matmul(
                        ps[:, col:col + 1],
                        E[:, j * TQ + qc * P: j * TQ + (qc + 1) * P],
                        wvec[:], start=True, stop=True)

        # ---------------- softmax (over k) + attn out + transpose ----------------
        attnT = cpool.tile([P, KN, TQ], F32)
        for qc in range(4):
            ps = attn_ps0 if qc < 2 else attn_ps1
            asl = ps[:, (qc % 2) * TK:(qc % 2 + 1) * TK]
            nmx = smx.tile([P, 1], F32, tag="nmx", name=f"nmx{qc}")
            nc.vector.reduce_max(nmx[:], asl, axis=AX.X, negate=True)
            ae = smx.tile([P, TK], F32, tag="ae", name=f"ae{qc}")
            nc.scalar.activation(ae[:], asl, AF.Exp, bias=nmx[:])
            sm = smx.tile([P, 1], F32, tag="sm", name=f"sm{qc}")
            nc.vector.reduce_sum(sm[:], ae[:], axis=AX.X)
            rc = smx.tile([P, 1], F32, tag="rc", name=f"rc{qc}")
            nc.vector.reciprocal(rc[:], sm[:])
            an = smx.tile([P, TK], F32, tag="an", name=f"an{qc}")
            nc.vector.tensor_scalar_mul(an[:], ae[:], rc[:])
            nc.sync.dma_start(attn_d[qc * P:(qc + 1) * P, :], an[:])
            for kc in range(KN):
                tp_a = psT.tile([P, P], F32, tag="tp", name=f"tp_a{qc}_{kc}")
                nc.tensor.transpose(tp_a[:], an[:, kc * P:(kc + 1) * P], ident[:])
                nc.vector.tensor_copy(attnT[:, kc, qc * P:(qc + 1) * P], tp_a[:])

        # ---------------- out = vprojT.T @ attnT  (C, TQ) ----------------
        out_ps = psP.tile([C, TQ], F32, tag="pp", name="out_ps")
        for kc in range(KN):
            nc.tensor.matmul(out_ps[:], vprojT[:, kc, :], attnT[:, kc, :],
                             start=(kc == 0), stop=(kc == KN - 1))
        out_sb = cpool.tile([C, TQ], F32)
        nc.vector.tensor_copy(out_sb[:], out_ps[:])
        nc.sync.dma_start(out_d[:], out_sb[:])


def build_nc(loop_n=None):
    nc = bacc.Bacc("TRN2", target_bir_lowering=False, debug=False,
                   enable_asserts=False, num_devices=N_CORES)
    q_d = nc.dram_tensor("queries", [TQ, NMEL], F32, kind="ExternalInput")
    k_d = nc.dram_tensor("keys", [TK, NCTX], F32, kind="ExternalInput")
    v_d = nc.dram_tensor("values", [TK, NCTX], F32, kind="ExternalInput")
    wq_d = nc.dram_tensor("Wq", [C, NMEL], F32, kind="ExternalInput")
    wk_d = nc.dram_tensor("Wk", [C, NCTX], F32, kind="ExternalInput")
    wv_d = nc.dram_tensor("Wv", [C, NCTX], F32, kind="ExternalInput")
    wvec_d = nc.dram_tensor("wvec16", [C, 1], F16, kind="ExternalInput")
    out_d = nc.dram_tensor("out", [C, TQ], F32, kind="ExternalOutput")
    attn_d = nc.dram_tensor("attn", [TQ, TK], F32, kind="ExternalOutput")

    ET = mybir.EngineType
    with tile.TileContext(nc) as tc:
        if loop_n is None:
            _emit(tc, nc, q_d, k_d, v_d, wq_d, wk_d, wv_d, wvec_d, out_d, attn_d)
        else:
            # timing mode: run the whole (idempotent) body loop_n times on-device
            with tc.For_i(0, loop_n, 1,
                          hint_engines=(ET.PE, ET.DVE, ET.Activation, ET.SP)):
                _emit(tc, nc, q_d, k_d, v_d, wq_d, wk_d, wv_d, wvec_d, out_d, attn_d)
    nc.compile()
    return nc


_NC = None


def _get_nc():
    global _NC
    if _NC is None:
        _NC = build_nc()
    return _NC


def make_in_maps(queries, keys, values, Wq, Wk, Wv, Wvec):
    wvec16 = np.ascontiguousarray(np.asarray(Wvec).astype(np.float16).reshape(C, 1))
    wq = np.ascontiguousarray(np.asarray(Wq, dtype=np.float32))
    wk = np.ascontiguousarray(np.asarray(Wk, dtype=np.float32))
    wv = np.ascontiguousarray(np.asarray(Wv, dtype=np.float32))
    in_maps = []
    for b in range(B):
        in_maps.append({
            "queries": np.ascontiguousarray(np.asarray(queries)[:, b, :], dtype=np.float32),
            "keys": np.ascontiguousarray(np.asarray(keys)[:, b, :], dtype=np.float32),
            "values": np.ascontiguousarray(np.asarray(values)[:, b, :], dtype=np.float32),
            "Wq": wq, "Wk": wk, "Wv": wv, "wvec16": wvec16,
        })
    return in_maps


def kernel(queries, keys, values, Wq, Wk, Wv, Wvec):
    from concourse.bass2jax import run_bass_via_pjrt
    nc = _get_nc()
    in_maps = make_in_maps(queries, keys, values, Wq, Wk, Wv, Wvec)
    results = run_bass_via_pjrt(nc, in_maps, n_cores=N_CORES)
    out = np.stack([results[b]["out"] for b in range(B)], axis=0)
    attn = np.stack([results[b]["attn"] for b in range(B)], axis=0)
    return out, attn
